# revision 74
# baseline (speedup 1.0000x reference)
"""Trainium2 Bass kernel for batched GCN (2x GCNConv + circular Conv1d).

Math per graph (N=64 nodes, S=96 feats, H=512 hidden, E=512 edges):
    deg[d]   = indegree + 1 (self loop)
    As       = Dinv (C + I) Dinv,  Dinv = diag(1/sqrt(deg)), C[d,s] counts
    h1       = relu((As X) W1^T + b1)          # aggregate-first (96-wide)
    h2       = As (h1 W2^T) + b2
    y        = circular_conv1d(h2, conv_w)     # emitted [o, l]-major

Device strategy (per core: 64 graphs = 32 pairs; pair nodes occupy
partition halves 0-63 / 64-127; pairs processed in groups - tapered
2,2,4..4,2,2 so pipeline fill/drain is short - to amortize per-op init
overheads on the elementwise engines):
  - edges host-transposed to [epos, (pair, chunk, j)] bf16; Pool
    materializes the edge broadcast (it may only TensorCopy/Memset on
    real hw), DVE runs is_equal in 2x mode against an iota table.
  - C built per graph with K=128 one-hot matmuls + identity matmul
    (tile_position quadrants); deg via one batched reduce per group.
  - As assembled block-diag [s, d]: row-scale dinv_d, one 128x128 PE
    transpose per pair (transpose outs must start at PSUM partition 0),
    row-scale dinv_s; both GCN normalizations live in the matrix.
  - layer1 aggregates x first (96-wide), then expands through W1 chunks
    transposed so layer2 needs no transposes.
  - conv as (gl,oc) units: 3 tap-matmuls spanning the whole group;
    output staged bf16, one DMA per group; host undoes layout + casts.
  - PSUM is bank-granular (8 x 2KB): tiles with disjoint lifetimes
    share banks (big = maug->mstb->tT, za = z2->a2) so every tag
    double-buffers and groups pipeline.
"""

import numpy as np
import ml_dtypes

import concourse.bacc as bacc
import concourse.mybir as mybir
import concourse.tile as tile
from concourse.bass_utils import run_bass_kernel_spmd

BF16 = mybir.dt.bfloat16
FP32 = mybir.dt.float32
AF = mybir.ActivationFunctionType
MUL = mybir.AluOpType.mult
ISEQ = mybir.AluOpType.is_equal

N_CORES = 8
B, S, N, H, E = 512, 96, 64, 512, 512
G = B // N_CORES          # graphs per core (64)
NPAIR = G // 2            # 32
GRP = 4                   # max pairs per group (tile sizing)
GROUPS = [2, 2, 4, 4, 4, 4, 4, 4, 2, 2]
assert sum(GROUPS) == NPAIR

SW = 256                  # oh cols per (pair, chunk)
PRW = 4 * SW              # oh cols per pair


def _phase_a(nc, P, q0, gs, has_b1, has_b2):
    """One-hots through tTs for a group; returns state for later phases."""
    (const, ohp, gsb, psb, gps, tps, pps, cps, xt, etr, w1t, w2t, cwd, cwp,
     i64d, iota, id1f, ones1, b1c, b2d, y_d) = P

    # ---- one-hots: oh[e, (pr, c, j, v)]  (a = pr*4+c merged) ----
    # Pool (the only engine free for it) replicates each edge id just 16x;
    # DVE compares the packed replica against 4 shifted iota slices in 2x
    # mode. This keeps Pool off the critical path (it was saturated when
    # it broadcast the full 64).
    na = 4 * gs
    erep = ohp.tile([128, GRP * 256], BF16, tag="erep")
    erv = erep[:, 0:gs * 256].rearrange("p (a j r) -> p a j r", a=na, j=4)
    oh = ohp.tile([128, GRP * PRW], BF16, tag="oh")
    ohv = oh[:, 0:gs * PRW].rearrange("p (a j v) -> p a j v", a=na, j=4)
    ev = etr[:, 16 * q0:16 * (q0 + gs)].rearrange("p (a j) -> p a j", j=4)
    e_all = ev.rearrange(
        "p a (j u) -> p a j u", u=1).to_broadcast([128, na, 4, 16])
    # per-2-pair slices so each pair's maug matmuls start without waiting
    # for the whole group's compares
    nsl = max(1, gs // 2)
    asl = na // nsl
    for s in range(nsl):
        a0 = slice(asl * s, asl * (s + 1))
        nc.gpsimd.tensor_copy(out=erv[:, a0], in_=e_all[:, a0])
        for qv in range(4):
            iota_r = iota[:, 16 * qv:16 * qv + 16].rearrange(
                "p (a j r) -> p a j r", a=1, j=1).to_broadcast(
                    [128, asl, 4, 16])
            nc.vector.tensor_tensor(
                out=ohv[:, a0, :, 16 * qv:16 * qv + 16],
                in0=erv[:, a0], in1=iota_r, op=ISEQ)

    # ---- maug: per graph C[d, s] + I ----
    big = gps.tile([128, 512], FP32, tag="big")
    maug = big[:, 0:64 * gs]
    for pg in range(gs):
        for gl in range(2):
            out_sl = maug[64 * gl:64 * gl + 64, 64 * pg:64 * pg + 64]
            tp = None if gl == 0 else (0, 64)
            for c in range(4):
                base = PRW * pg + SW * c
                lhsT = oh[:, base + 128 + 64 * gl:base + 192 + 64 * gl]
                rhs = oh[:, base + 64 * gl:base + 64 * gl + 64]
                nc.tensor.matmul(out_sl, lhsT, rhs, start=(c == 0),
                                 stop=False, tile_position=tp)
            nc.tensor.matmul(
                out_sl, i64d[64 * gl:64 * gl + 64, :],
                i64d[64 * gl:64 * gl + 64, :],
                start=False, stop=True,
                tile_position=None if gl == 0 else (64, 64),
            )

    mv = maug.rearrange("p (pr v) -> p pr v", pr=gs)

    # ---- deg via 1-col matmuls (indegree; +1 via sqrt bias) into the
    #      spare columns of the za bank ----
    za = tps.tile([128, 512], FP32, tag="za")
    degp = za[:, GRP * 96:GRP * 96 + gs]
    for pg in range(gs):
        for gl in range(2):
            for c in range(4):
                base = PRW * pg + SW * c
                nc.tensor.matmul(
                    degp[64 * gl:64 * gl + 64, pg:pg + 1],
                    oh[:, base + 128 + 64 * gl:base + 192 + 64 * gl],
                    ones1[:],
                    start=(c == 0), stop=(c == 3),
                    tile_position=None if gl == 0 else (0, 64),
                )
    sq_t = gsb.tile([128, GRP], FP32, tag="sq")
    sq = sq_t[:, 0:gs]
    nc.scalar.activation(out=sq, in_=degp, func=AF.Sqrt, bias=1.0)
    dinv_t = gsb.tile([128, GRP], FP32, tag="dinv")
    dinv = dinv_t[:, 0:gs]
    nc.vector.reciprocal(out=dinv, in_=sq)

    # ---- msb = dinv_d * C, block-diag per pair ----
    msb = gsb.tile([128, GRP * 128], FP32, tag="msb")
    nc.gpsimd.memset(msb[:, 0:128 * gs], 0)
    for gl in range(2):
        h = slice(64 * gl, 64 * gl + 64)
        din_h = dinv[h, :].rearrange(
            "p (pr u) -> p pr u", u=1).to_broadcast([64, gs, 64])
        nc.vector.tensor_tensor(
            out=msb[h, 0:128 * gs].rearrange(
                "p (pr v) -> p pr v", pr=gs)[:, :, 64 * gl:64 * gl + 64],
            in0=maug[h, :].rearrange("p (pr v) -> p pr v", pr=gs),
            in1=din_h, op=MUL)

    # ---- transpose per pair (fp32) -> mstb block-diag (reuses maug bank)
    mstb = big[:, 0:128 * gs]
    for pg in range(gs):
        sl = slice(128 * pg, 128 * pg + 128)
        nc.tensor.transpose(out=mstb[:, sl], in_=msb[:, sl],
                            identity=id1f[:])

    # ---- msbd = dinv_s * mstb  ( = As as [s, d] ) ----
    msbd_t = gsb.tile([128, GRP * 128], BF16, tag="msbd")
    msbd = msbd_t[:, 0:128 * gs]
    dinv_b128 = dinv[:, :].rearrange(
        "p (pr u) -> p pr u", u=1).to_broadcast([128, gs, 128])
    nc.vector.tensor_tensor(
        out=msbd.rearrange("p (pr v) -> p pr v", pr=gs),
        in0=mstb.rearrange("p (pr v) -> p pr v", pr=gs),
        in1=dinv_b128, op=MUL)

    # ---- tT[f, d] (pair-packed; reuses the same bank after msbd) ----
    tT = big[0:96, 0:128 * gs]
    for pg in range(gs):
        nc.tensor.matmul(
            tT[:, 128 * pg:128 * pg + 128],
            xt[:, 96 * (q0 + pg):96 * (q0 + pg) + 96],
            msbd[:, 128 * pg:128 * pg + 128],
            start=True, stop=True,
        )
    tTs_t = gsb.tile([96, GRP * 128], BF16, tag="tTs")
    tTs = tTs_t[:, 0:128 * gs]
    nc.scalar.activation(out=tTs, in_=tT, func=AF.Copy)
    return dict(za=za, msbd=msbd, tTs=tTs)


def _phase_b(nc, P, st, q0, gs, has_b1, has_b2):
    """Per-pair pre1/relu/z2 chain + z2s."""
    (const, ohp, gsb, psb, gps, tps, pps, cps, xt, etr, w1t, w2t, cwd, cwp,
     i64d, iota, id1f, ones1, b1c, b2d, y_d) = P
    za, tTs = st["za"], st["tTs"]

    z2 = za[:, 0:96 * gs]
    for pg in range(gs):
        pre1 = pps.tile([128, 512], FP32, tag="pre1")
        for c in range(4):
            nc.tensor.matmul(pre1[:, 128 * c:128 * c + 128],
                             w1t[:, 128 * c:128 * c + 128],
                             tTs[:, 128 * pg:128 * pg + 128],
                             start=True, stop=True)
        h1t = psb.tile([128, 512], BF16, tag="h1t")
        if has_b1:
            for c in range(4):
                nc.scalar.activation(
                    out=h1t[:, 128 * c:128 * c + 128],
                    in_=pre1[:, 128 * c:128 * c + 128],
                    func=AF.Relu, bias=b1c[:, c:c + 1])
            for c in range(4):
                nc.tensor.matmul(z2[:, 96 * pg:96 * pg + 96],
                                 h1t[:, 128 * c:128 * c + 128],
                                 w2t[:, 96 * c:96 * c + 96],
                                 start=(c == 0), stop=(c == 3))
        else:
            nc.scalar.activation(out=h1t[:], in_=pre1[:], func=AF.Relu)
            for c in range(4):
                nc.tensor.matmul(z2[:, 96 * pg:96 * pg + 96],
                                 h1t[:, 128 * c:128 * c + 128],
                                 w2t[:, 96 * c:96 * c + 96],
                                 start=(c == 0), stop=(c == 3))

    # (gpsimd cannot access PSUM on real hw - keep off Pool)
    z2s_t = gsb.tile([128, GRP * 96], BF16, tag="z2s")
    z2s = z2s_t[:, 0:96 * gs]
    nc.vector.tensor_copy(out=z2s, in_=z2)
    st["z2s"] = z2s


def _phase_c(nc, P, st, q0, gs, has_b1, has_b2):
    """a2, hp, hs2, conv, output staging and DMA."""
    (const, ohp, gsb, psb, gps, tps, pps, cps, xt, etr, w1t, w2t, cwd, cwp,
     i64d, iota, id1f, ones1, b1c, b2d, y_d) = P
    za, msbd, z2s = st["za"], st["msbd"], st["z2s"]

    hp_t = gsb.tile([128, GRP * 192], BF16, tag="hp")
    hp = hp_t[:, 0:192 * gs]
    a2 = za[:, 0:96 * gs]
    gh = gs
    for sh in range(0, gs, gh):
        psl = slice(96 * sh, 96 * (sh + gh))
        for pg in range(sh, sh + gh):
            for gl in range(2):
                h = slice(64 * gl, 64 * gl + 64)
                nc.tensor.matmul(
                    a2[h, 96 * pg:96 * pg + 96],
                    msbd[h, 128 * pg + 64 * gl:128 * pg + 64 * gl + 64],
                    z2s[h, 96 * pg:96 * pg + 96],
                    start=True, stop=True,
                    tile_position=None if gl == 0 else (64, 64),
                )
        a2_b = a2[:, psl].rearrange("p (pr t v) -> p pr t v", pr=gh,
                                    t=1).to_broadcast([128, gh, 2, 96])
        hpv = hp[:, 192 * sh:192 * (sh + gh)].rearrange(
            "p (pr t v) -> p pr t v", pr=gh, t=2)
        nc.scalar.activation(out=hpv, in_=a2_b, func=AF.Copy)
    if has_b2:
        hpb_t = gsb.tile([128, GRP * 192], BF16, tag="hpb")
        hpb = hpb_t[:, 0:192 * gs]
        nc.vector.tensor_tensor(
            out=hpb.rearrange("p (pr v) -> p pr v", pr=gs),
            in0=hp.rearrange("p (pr v) -> p pr v", pr=gs),
            in1=b2d[:].rearrange("p (t v) -> p t v", t=1).to_broadcast(
                [128, gs, 192]),
            op=mybir.AluOpType.add)
        hp = hpb

    # ---- conv: taps k=0,1 packed on the partition axis (hs2, built by
    #      SBUF->SBUF DMA from hp), + a k=2 accumulate matmul ----
    # (k, gl) regions where partitions are unchanged go through Pool (it
    # has slack); the two partition-crossing ones go through DMA
    hs2_t = psb.tile([128, GRP * 192], BF16, tag="hs2")
    for k in range(2):
        tap = (95, 0)[k]
        for gl in range(2):
            out_ap = hs2_t[64 * k:64 * k + 64,
                           96 * gs * gl:96 * gs * gl + 96 * gs].rearrange(
                               "p (pr l) -> p pr l", pr=gs)
            in_ap = hp[64 * gl:64 * gl + 64, :].rearrange(
                "p (pr v) -> p pr v", pr=gs)[:, :, tap:tap + 96]
            if k == gl:
                nc.gpsimd.tensor_copy(out=out_ap, in_=in_ap)
            else:
                nc.sync.dma_start(out=out_ap, in_=in_ap)
    ysb = psb.tile([128, GRP * 768], BF16, tag="ysb")
    ysv = ysb[:, 0:768 * gs].rearrange("p (pr r) -> p pr r", pr=gs)
    for gl in range(2):
        h = slice(64 * gl, 64 * gl + 64)
        for oc in range(4):
            yp = cps.tile([128, 384], FP32, tag="yp")
            ypv = yp[:, 0:96 * gs].rearrange("p (pr v) -> p pr v", pr=gs)
            nc.tensor.matmul(
                ypv, cwp[:, 128 * oc:128 * oc + 128],
                hs2_t[:, 96 * gs * gl:96 * gs * gl + 96 * gs].rearrange(
                    "p (pr l) -> p pr l", pr=gs),
                start=True, stop=False)
            nc.tensor.matmul(
                ypv, cwd[h, 128 * (8 + oc):128 * (8 + oc) + 128],
                hp[64 * gl:64 * gl + 64, :].rearrange(
                    "p (pr v) -> p pr v", pr=gs)[:, :, 1:97],
                start=False, stop=True)
            co = 384 * gl + 96 * oc
            if (gl + oc) % 2 == 0:
                nc.scalar.activation(out=ysv[:, :, co:co + 96], in_=ypv,
                                     func=AF.Copy)
            else:
                nc.vector.tensor_copy(out=ysv[:, :, co:co + 96], in_=ypv)
    nc.sync.dma_start(
        out=y_d[q0:q0 + gs].rearrange("g p v -> p g v"),
        in_=ysb[:, 0:768 * gs].rearrange("p (g v) -> p g v", g=gs))


def build_gcn_kernel(tc, outs, ins, has_b1=False, has_b2=False):
    nc = tc.nc
    y_d = outs["y"]         # [32, 128, 768] bf16

    from contextlib import ExitStack
    ctx = ExitStack()
    const = ctx.enter_context(tc.tile_pool(name="const", bufs=1))
    ohp = ctx.enter_context(tc.tile_pool(name="ohp", bufs=4))
    gsb = ctx.enter_context(tc.tile_pool(name="gsb", bufs=4))
    psb = ctx.enter_context(tc.tile_pool(name="psb", bufs=6))
    gps = ctx.enter_context(tc.tile_pool(name="gps", bufs=2, space="PSUM"))
    tps = ctx.enter_context(tc.tile_pool(name="tps", bufs=2, space="PSUM"))
    pps = ctx.enter_context(tc.tile_pool(name="pps", bufs=2, space="PSUM"))
    cps = ctx.enter_context(tc.tile_pool(name="cps", bufs=2, space="PSUM"))

    # ---- constants (edge/iota first: they gate the pipeline head) ----
    etr = const.tile([128, 512], BF16)
    nc.sync.dma_start(out=etr[:], in_=ins["etr"][:])
    iota = const.tile([128, 64], BF16)
    nc.sync.dma_start(out=iota[:], in_=ins["iota"][:])
    i64d = const.tile([128, 64], BF16)
    nc.sync.dma_start(out=i64d[:], in_=ins["i64d"][:])
    id1f = const.tile([128, 128], FP32)
    nc.sync.dma_start(out=id1f[:], in_=ins["id1f"][:])
    xt = const.tile([128, 32 * 96], BF16)
    nc.sync.dma_start(out=xt[:], in_=ins["xt"][:])
    w1t = const.tile([96, 512], BF16)
    nc.sync.dma_start(out=w1t[:], in_=ins["w1t"][:])
    w2t = const.tile([128, 384], BF16)
    nc.sync.dma_start(out=w2t[:], in_=ins["w2t"][:])
    cwd = const.tile([128, 1536], BF16)
    nc.sync.dma_start(out=cwd[:], in_=ins["cwd"][:])
    cwp = const.tile([128, 512], BF16)
    nc.sync.dma_start(out=cwp[:], in_=ins["cwp"][:])
    ones1 = const.tile([128, 1], BF16)
    nc.gpsimd.memset(ones1[:], 1)
    b1c = b2d = None
    if has_b1:
        b1c = const.tile([128, 4], FP32)
        nc.sync.dma_start(out=b1c[:], in_=ins["b1c"][:])
    if has_b2:
        b2d = const.tile([128, 192], BF16)
        nc.sync.dma_start(out=b2d[:], in_=ins["b2d"][:])

    P = (const, ohp, gsb, psb, gps, tps, pps, cps, xt, etr, w1t, w2t, cwd,
         cwp, i64d, iota, id1f, ones1, b1c, b2d, y_d)
    # software-pipelined emission: A(q+2) | B(q+1) | C(q) so each engine's
    # in-order stream interleaves phases of different groups
    n = len(GROUPS)
    q0s = [sum(GROUPS[:i]) for i in range(n)]
    st = [None] * n
    for i in range(n + 2):
        if i < n:
            st[i] = _phase_a(nc, P, q0s[i], GROUPS[i], has_b1, has_b2)
        if 1 <= i <= n:
            _phase_b(nc, P, st[i - 1], q0s[i - 1], GROUPS[i - 1],
                     has_b1, has_b2)
        if i >= 2:
            _phase_c(nc, P, st[i - 2], q0s[i - 2], GROUPS[i - 2],
                     has_b1, has_b2)

    ctx.close()


# ---------------- host side ----------------

def _prep_consts(W1, b1, W2, b2, conv_w):
    bf = ml_dtypes.bfloat16
    w1t = np.ascontiguousarray(W1.T).astype(bf)                    # [96, 512]
    w2t = np.ascontiguousarray(
        W2.T.reshape(4, 128, 96).transpose(1, 0, 2).reshape(128, 384)
    ).astype(bf)
    # cwd[i, (k, oc, o_lo)] = conv_w[oc*128+o_lo, i, k], duplicated rows
    base = np.ascontiguousarray(
        conv_w.transpose(1, 2, 0).reshape(64, 3 * 4 * 128))
    cwd = np.concatenate([base, base], axis=0).astype(bf)          # [128,1536]
    i64 = np.eye(64)
    i64d = np.concatenate([i64, i64], axis=0).astype(bf)           # [128, 64]
    iota = np.ascontiguousarray(
        np.broadcast_to(np.arange(64).astype(bf), (128, 64)))
    id1f = np.eye(128, dtype=np.float32)
    # cwp: taps k=0,1 stacked on rows for the packed conv matmul
    ckio = conv_w.transpose(1, 2, 0)                               # [i, k, o]
    cwp = np.concatenate([ckio[:, 0, :], ckio[:, 1, :]], axis=0).astype(bf)
    consts = dict(w1t=w1t, w2t=w2t, cwd=cwd, cwp=cwp, i64d=i64d, iota=iota,
                  id1f=id1f)
    has_b1 = bool(np.any(b1))
    has_b2 = bool(np.any(b2))
    if has_b1:
        consts["b1c"] = np.ascontiguousarray(
            b1.reshape(4, 128).T).astype(np.float32)
    if has_b2:
        b2d = np.ascontiguousarray(
            np.broadcast_to(np.tile(b2, 2).astype(bf), (128, 192)))
        consts["b2d"] = b2d
    return consts, has_b1, has_b2


_NC_CACHE = {}


def _get_nc(has_b1, has_b2):
    key = (has_b1, has_b2)
    if key in _NC_CACHE:
        return _NC_CACHE[key]
    nc = bacc.Bacc("TRN2", target_bir_lowering=False, debug=False)
    ins = {
        "xt": nc.dram_tensor("xt", [128, 32 * 96], BF16,
                             kind="ExternalInput").ap(),
        "etr": nc.dram_tensor("etr", [128, 512], BF16,
                              kind="ExternalInput").ap(),
        "w1t": nc.dram_tensor("w1t", [96, 512], BF16,
                              kind="ExternalInput").ap(),
        "w2t": nc.dram_tensor("w2t", [128, 384], BF16,
                              kind="ExternalInput").ap(),
        "cwd": nc.dram_tensor("cwd", [128, 1536], BF16,
                              kind="ExternalInput").ap(),
        "i64d": nc.dram_tensor("i64d", [128, 64], BF16,
                               kind="ExternalInput").ap(),
        "iota": nc.dram_tensor("iota", [128, 64], BF16,
                               kind="ExternalInput").ap(),
        "id1f": nc.dram_tensor("id1f", [128, 128], FP32,
                               kind="ExternalInput").ap(),
        "cwp": nc.dram_tensor("cwp", [128, 512], BF16,
                              kind="ExternalInput").ap(),
    }
    if has_b1:
        ins["b1c"] = nc.dram_tensor("b1c", [128, 4], FP32,
                                    kind="ExternalInput").ap()
    if has_b2:
        ins["b2d"] = nc.dram_tensor("b2d", [128, 192], BF16,
                                    kind="ExternalInput").ap()
    outs = {
        "y": nc.dram_tensor("y", [NPAIR, 128, 768], BF16,
                            kind="ExternalOutput").ap(),
    }
    with tile.TileContext(nc) as tc:
        build_gcn_kernel(tc, outs, ins, has_b1, has_b2)
    nc.compile()
    _NC_CACHE[key] = nc
    return nc


def kernel(x, edge_index, W1, b1, W2, b2, conv_w, _trace=False):
    bf = ml_dtypes.bfloat16
    x = np.asarray(x)
    edge_index = np.asarray(edge_index)
    consts, has_b1, has_b2 = _prep_consts(
        np.asarray(W1), np.asarray(b1), np.asarray(W2), np.asarray(b2),
        np.asarray(conv_w))
    nc = _get_nc(has_b1, has_b2)

    in_maps = []
    for cid in range(N_CORES):
        sl = slice(cid * G, (cid + 1) * G)
        m = dict(consts)
        xc = np.asarray(x[sl])                       # [64, 96, 64]
        m["xt"] = np.ascontiguousarray(
            xc.reshape(32, 2, 96, 64).transpose(1, 3, 0, 2).reshape(
                128, 32 * 96)).astype(bf)
        ec = np.asarray(edge_index[sl])              # [64, 2, 512]
        m["etr"] = np.ascontiguousarray(
            ec.reshape(32, 2, 2, 4, 128).transpose(4, 0, 3, 2, 1).reshape(
                128, 512)).astype(bf)
        in_maps.append(m)

    res = run_bass_kernel_spmd(nc, in_maps, core_ids=list(range(N_CORES)),
                               trace=_trace)
    parts = []
    for cid in range(N_CORES):
        arr = np.asarray(res.results[cid]["y"])      # [32, 128, 768] bf16
        yc = arr.reshape(32, 128, 2, 4, 96).transpose(0, 2, 4, 3, 1)
        parts.append(yc.reshape(G, 96, 512).astype(np.float32))
    y = np.concatenate(parts, axis=0)
    if _trace:
        kernel.last_results = res
    return y


# revision 75
# speedup vs baseline: 1.0269x; 1.0269x over previous
"""Trainium2 Bass kernel for batched GCN (2x GCNConv + circular Conv1d).

Math per graph (N=64 nodes, S=96 feats, H=512 hidden, E=512 edges):
    deg[d]   = indegree + 1 (self loop)
    As       = Dinv (C + I) Dinv,  Dinv = diag(1/sqrt(deg)), C[d,s] counts
    h1       = relu((As X) W1^T + b1)          # aggregate-first (96-wide)
    h2       = As (h1 W2^T) + b2
    y        = circular_conv1d(h2, conv_w)     # emitted [o, l]-major

Device strategy (per core: 64 graphs = 32 pairs; pair nodes occupy
partition halves 0-63 / 64-127; pairs processed in groups - tapered
2,2,4..4,2,2 so pipeline fill/drain is short - to amortize per-op init
overheads on the elementwise engines):
  - edges host-transposed to [epos, (pair, chunk, j)] bf16; Pool
    materializes the edge broadcast (it may only TensorCopy/Memset on
    real hw), DVE runs is_equal in 2x mode against an iota table.
  - C built per graph with K=128 one-hot matmuls + identity matmul
    (tile_position quadrants); deg via one batched reduce per group.
  - As assembled block-diag [s, d]: row-scale dinv_d, one 128x128 PE
    transpose per pair (transpose outs must start at PSUM partition 0),
    row-scale dinv_s; both GCN normalizations live in the matrix.
  - layer1 aggregates x first (96-wide), then expands through W1 chunks
    transposed so layer2 needs no transposes.
  - conv as (gl,oc) units: 3 tap-matmuls spanning the whole group;
    output staged bf16, one DMA per group; host undoes layout + casts.
  - PSUM is bank-granular (8 x 2KB): tiles with disjoint lifetimes
    share banks (big = maug->mstb->tT, za = z2->a2) so every tag
    double-buffers and groups pipeline.
"""

import numpy as np
import ml_dtypes

import concourse.bacc as bacc
import concourse.mybir as mybir
import concourse.tile as tile
from concourse.bass_utils import run_bass_kernel_spmd

BF16 = mybir.dt.bfloat16
FP32 = mybir.dt.float32
AF = mybir.ActivationFunctionType
MUL = mybir.AluOpType.mult
ISEQ = mybir.AluOpType.is_equal

N_CORES = 8
B, S, N, H, E = 512, 96, 64, 512, 512
G = B // N_CORES          # graphs per core (64)
NPAIR = G // 2            # 32
GRP = 4                   # max pairs per group (tile sizing)
GROUPS = [2, 2, 4, 4, 4, 4, 4, 4, 2, 2]
assert sum(GROUPS) == NPAIR

SW = 256                  # oh cols per (pair, chunk)
PRW = 4 * SW              # oh cols per pair


def _phase_a(nc, P, q0, gs, has_b1, has_b2):
    """One-hots through tTs for a group; returns state for later phases."""
    (const, ohp, gsb, psb, gps, tps, pps, cps, xt, etr, w1t, w2t, cwd, cwp,
     i64d, iota, id1f, ones1, b1c, b2d, y_d) = P

    # ---- one-hots: oh[e, (pr, c, j, v)]  (a = pr*4+c merged) ----
    # Pool (the only engine free for it) replicates each edge id just 16x;
    # DVE compares the packed replica against 4 shifted iota slices in 2x
    # mode. This keeps Pool off the critical path (it was saturated when
    # it broadcast the full 64).
    na = 4 * gs
    erep = ohp.tile([128, GRP * 256], BF16, tag="erep")
    erv = erep[:, 0:gs * 256].rearrange("p (a j r) -> p a j r", a=na, j=4)
    oh = ohp.tile([128, GRP * PRW], BF16, tag="oh")
    ohv = oh[:, 0:gs * PRW].rearrange("p (a j v) -> p a j v", a=na, j=4)
    ev = etr[:, 16 * q0:16 * (q0 + gs)].rearrange("p (a j) -> p a j", j=4)
    e_all = ev.rearrange(
        "p a (j u) -> p a j u", u=1).to_broadcast([128, na, 4, 16])
    # per-2-pair slices so each pair's maug matmuls start without waiting
    # for the whole group's compares
    nsl = max(1, gs // 2)
    asl = na // nsl
    for s in range(nsl):
        a0 = slice(asl * s, asl * (s + 1))
        nc.gpsimd.tensor_copy(out=erv[:, a0], in_=e_all[:, a0])
        for qv in range(4):
            iota_r = iota[:, 16 * qv:16 * qv + 16].rearrange(
                "p (a j r) -> p a j r", a=1, j=1).to_broadcast(
                    [128, asl, 4, 16])
            nc.vector.tensor_tensor(
                out=ohv[:, a0, :, 16 * qv:16 * qv + 16],
                in0=erv[:, a0], in1=iota_r, op=ISEQ)

    # ---- maug: per graph C[d, s] + I ----
    big = gps.tile([128, 512], FP32, tag="big")
    maug = big[:, 0:64 * gs]
    for pg in range(gs):
        for gl in range(2):
            out_sl = maug[64 * gl:64 * gl + 64, 64 * pg:64 * pg + 64]
            tp = None if gl == 0 else (0, 64)
            for c in range(4):
                base = PRW * pg + SW * c
                lhsT = oh[:, base + 128 + 64 * gl:base + 192 + 64 * gl]
                rhs = oh[:, base + 64 * gl:base + 64 * gl + 64]
                nc.tensor.matmul(out_sl, lhsT, rhs, start=(c == 0),
                                 stop=False, tile_position=tp)
            nc.tensor.matmul(
                out_sl, i64d[64 * gl:64 * gl + 64, :],
                i64d[64 * gl:64 * gl + 64, :],
                start=False, stop=True,
                tile_position=None if gl == 0 else (64, 64),
            )

    mv = maug.rearrange("p (pr v) -> p pr v", pr=gs)

    # ---- deg via 1-col matmuls (indegree; +1 via sqrt bias) into the
    #      spare columns of the za bank ----
    za = tps.tile([128, 512], FP32, tag="za")
    degp = za[:, GRP * 96:GRP * 96 + gs]
    for pg in range(gs):
        for gl in range(2):
            for c in range(4):
                base = PRW * pg + SW * c
                nc.tensor.matmul(
                    degp[64 * gl:64 * gl + 64, pg:pg + 1],
                    oh[:, base + 128 + 64 * gl:base + 192 + 64 * gl],
                    ones1[:],
                    start=(c == 0), stop=(c == 3),
                    tile_position=None if gl == 0 else (0, 64),
                )
    sq_t = gsb.tile([128, GRP], FP32, tag="sq")
    sq = sq_t[:, 0:gs]
    nc.scalar.activation(out=sq, in_=degp, func=AF.Sqrt, bias=1.0)
    dinv_t = gsb.tile([128, GRP], FP32, tag="dinv")
    dinv = dinv_t[:, 0:gs]
    nc.vector.reciprocal(out=dinv, in_=sq)

    # ---- msb = dinv_d * C, block-diag per pair ----
    msb = gsb.tile([128, GRP * 128], FP32, tag="msb")
    nc.gpsimd.memset(msb[:, 0:128 * gs], 0)
    for gl in range(2):
        h = slice(64 * gl, 64 * gl + 64)
        din_h = dinv[h, :].rearrange(
            "p (pr u) -> p pr u", u=1).to_broadcast([64, gs, 64])
        nc.vector.tensor_tensor(
            out=msb[h, 0:128 * gs].rearrange(
                "p (pr v) -> p pr v", pr=gs)[:, :, 64 * gl:64 * gl + 64],
            in0=maug[h, :].rearrange("p (pr v) -> p pr v", pr=gs),
            in1=din_h, op=MUL)

    # ---- transpose per pair (fp32) -> mstb block-diag (reuses maug bank)
    mstb = big[:, 0:128 * gs]
    for pg in range(gs):
        sl = slice(128 * pg, 128 * pg + 128)
        nc.tensor.transpose(out=mstb[:, sl], in_=msb[:, sl],
                            identity=id1f[:])

    # ---- msbd = dinv_s * mstb  ( = As as [s, d] ) ----
    msbd_t = gsb.tile([128, GRP * 128], BF16, tag="msbd")
    msbd = msbd_t[:, 0:128 * gs]
    dinv_b128 = dinv[:, :].rearrange(
        "p (pr u) -> p pr u", u=1).to_broadcast([128, gs, 128])
    nc.vector.tensor_tensor(
        out=msbd.rearrange("p (pr v) -> p pr v", pr=gs),
        in0=mstb.rearrange("p (pr v) -> p pr v", pr=gs),
        in1=dinv_b128, op=MUL)

    # ---- tT[f, d] (pair-packed; reuses the same bank after msbd) ----
    tT = big[0:96, 0:128 * gs]
    for pg in range(gs):
        nc.tensor.matmul(
            tT[:, 128 * pg:128 * pg + 128],
            xt[:, 96 * (q0 + pg):96 * (q0 + pg) + 96],
            msbd[:, 128 * pg:128 * pg + 128],
            start=True, stop=True,
        )
    tTs_t = gsb.tile([96, GRP * 128], BF16, tag="tTs")
    tTs = tTs_t[:, 0:128 * gs]
    nc.scalar.activation(out=tTs, in_=tT, func=AF.Copy)
    return dict(za=za, msbd=msbd, tTs=tTs)


def _phase_b(nc, P, st, q0, gs, has_b1, has_b2):
    """Per-pair pre1/relu/z2 chain + z2s."""
    (const, ohp, gsb, psb, gps, tps, pps, cps, xt, etr, w1t, w2t, cwd, cwp,
     i64d, iota, id1f, ones1, b1c, b2d, y_d) = P
    za, tTs = st["za"], st["tTs"]

    z2 = za[:, 0:96 * gs]
    for pg in range(gs):
        pre1 = pps.tile([128, 512], FP32, tag="pre1")
        for c in range(4):
            nc.tensor.matmul(pre1[:, 128 * c:128 * c + 128],
                             w1t[:, 128 * c:128 * c + 128],
                             tTs[:, 128 * pg:128 * pg + 128],
                             start=True, stop=True)
        h1t = psb.tile([128, 512], BF16, tag="h1t")
        if has_b1:
            for c in range(4):
                nc.scalar.activation(
                    out=h1t[:, 128 * c:128 * c + 128],
                    in_=pre1[:, 128 * c:128 * c + 128],
                    func=AF.Relu, bias=b1c[:, c:c + 1])
            for c in range(4):
                nc.tensor.matmul(z2[:, 96 * pg:96 * pg + 96],
                                 h1t[:, 128 * c:128 * c + 128],
                                 w2t[:, 96 * c:96 * c + 96],
                                 start=(c == 0), stop=(c == 3))
        else:
            nc.scalar.activation(out=h1t[:], in_=pre1[:], func=AF.Relu)
            for c in range(4):
                nc.tensor.matmul(z2[:, 96 * pg:96 * pg + 96],
                                 h1t[:, 128 * c:128 * c + 128],
                                 w2t[:, 96 * c:96 * c + 96],
                                 start=(c == 0), stop=(c == 3))

    # (gpsimd cannot access PSUM on real hw - keep off Pool)
    z2s_t = gsb.tile([128, GRP * 96], BF16, tag="z2s")
    z2s = z2s_t[:, 0:96 * gs]
    nc.vector.tensor_copy(out=z2s, in_=z2)
    st["z2s"] = z2s


def _phase_c(nc, P, st, q0, gs, has_b1, has_b2):
    """a2, hp, hs2, conv, output staging and DMA."""
    (const, ohp, gsb, psb, gps, tps, pps, cps, xt, etr, w1t, w2t, cwd, cwp,
     i64d, iota, id1f, ones1, b1c, b2d, y_d) = P
    za, msbd, z2s = st["za"], st["msbd"], st["z2s"]

    hp_t = gsb.tile([128, GRP * 192], BF16, tag="hp")
    hp = hp_t[:, 0:192 * gs]
    a2 = za[:, 0:96 * gs]
    gh = gs
    for sh in range(0, gs, gh):
        psl = slice(96 * sh, 96 * (sh + gh))
        for pg in range(sh, sh + gh):
            for gl in range(2):
                h = slice(64 * gl, 64 * gl + 64)
                nc.tensor.matmul(
                    a2[h, 96 * pg:96 * pg + 96],
                    msbd[h, 128 * pg + 64 * gl:128 * pg + 64 * gl + 64],
                    z2s[h, 96 * pg:96 * pg + 96],
                    start=True, stop=True,
                    tile_position=None if gl == 0 else (64, 64),
                )
        a2_b = a2[:, psl].rearrange("p (pr t v) -> p pr t v", pr=gh,
                                    t=1).to_broadcast([128, gh, 2, 96])
        hpv = hp[:, 192 * sh:192 * (sh + gh)].rearrange(
            "p (pr t v) -> p pr t v", pr=gh, t=2)
        nc.scalar.activation(out=hpv, in_=a2_b, func=AF.Copy)
    if has_b2:
        hpb_t = gsb.tile([128, GRP * 192], BF16, tag="hpb")
        hpb = hpb_t[:, 0:192 * gs]
        nc.vector.tensor_tensor(
            out=hpb.rearrange("p (pr v) -> p pr v", pr=gs),
            in0=hp.rearrange("p (pr v) -> p pr v", pr=gs),
            in1=b2d[:].rearrange("p (t v) -> p t v", t=1).to_broadcast(
                [128, gs, 192]),
            op=mybir.AluOpType.add)
        hp = hpb

    # ---- conv: taps k=0,1 packed on the partition axis (hs2, built by
    #      SBUF->SBUF DMA from hp), + a k=2 accumulate matmul ----
    # (k, gl) regions where partitions are unchanged go through Pool (it
    # has slack); the two partition-crossing ones go through DMA
    hs2_t = psb.tile([128, GRP * 192], BF16, tag="hs2")
    for k in range(2):
        tap = (95, 0)[k]
        for gl in range(2):
            out_ap = hs2_t[64 * k:64 * k + 64,
                           96 * gs * gl:96 * gs * gl + 96 * gs].rearrange(
                               "p (pr l) -> p pr l", pr=gs)
            in_ap = hp[64 * gl:64 * gl + 64, :].rearrange(
                "p (pr v) -> p pr v", pr=gs)[:, :, tap:tap + 96]
            if k == gl:
                nc.gpsimd.tensor_copy(out=out_ap, in_=in_ap)
            else:
                nc.sync.dma_start(out=out_ap, in_=in_ap)
    ysb = psb.tile([128, GRP * 768], BF16, tag="ysb")
    ysv = ysb[:, 0:768 * gs].rearrange("p (pr r) -> p pr r", pr=gs)
    for gl in range(2):
        h = slice(64 * gl, 64 * gl + 64)
        for oc in range(4):
            yp = cps.tile([128, 384], FP32, tag="yp")
            ypv = yp[:, 0:96 * gs].rearrange("p (pr v) -> p pr v", pr=gs)
            nc.tensor.matmul(
                ypv, cwp[:, 128 * oc:128 * oc + 128],
                hs2_t[:, 96 * gs * gl:96 * gs * gl + 96 * gs].rearrange(
                    "p (pr l) -> p pr l", pr=gs),
                start=True, stop=False)
            nc.tensor.matmul(
                ypv, cwd[h, 128 * (8 + oc):128 * (8 + oc) + 128],
                hp[64 * gl:64 * gl + 64, :].rearrange(
                    "p (pr v) -> p pr v", pr=gs)[:, :, 1:97],
                start=False, stop=True)
            co = 384 * gl + 96 * oc
            if (gl + oc) % 2 == 0:
                nc.scalar.activation(out=ysv[:, :, co:co + 96], in_=ypv,
                                     func=AF.Copy)
            else:
                nc.vector.tensor_copy(out=ysv[:, :, co:co + 96], in_=ypv)
    nc.sync.dma_start(
        out=y_d[q0:q0 + gs].rearrange("g p v -> p g v"),
        in_=ysb[:, 0:768 * gs].rearrange("p (g v) -> p g v", g=gs))


def build_gcn_kernel(tc, outs, ins, has_b1=False, has_b2=False):
    nc = tc.nc
    y_d = outs["y"]         # [32, 128, 768] bf16

    from contextlib import ExitStack
    ctx = ExitStack()
    const = ctx.enter_context(tc.tile_pool(name="const", bufs=1))
    ohp = ctx.enter_context(tc.tile_pool(name="ohp", bufs=4))
    gsb = ctx.enter_context(tc.tile_pool(name="gsb", bufs=4))
    psb = ctx.enter_context(tc.tile_pool(name="psb", bufs=6))
    gps = ctx.enter_context(tc.tile_pool(name="gps", bufs=2, space="PSUM"))
    tps = ctx.enter_context(tc.tile_pool(name="tps", bufs=2, space="PSUM"))
    pps = ctx.enter_context(tc.tile_pool(name="pps", bufs=2, space="PSUM"))
    cps = ctx.enter_context(tc.tile_pool(name="cps", bufs=2, space="PSUM"))

    # ---- constants (edge/iota first: they gate the pipeline head) ----
    etr = const.tile([128, 512], BF16)
    nc.sync.dma_start(out=etr[:], in_=ins["etr"][:])
    iota = const.tile([128, 64], BF16)
    nc.sync.dma_start(out=iota[:], in_=ins["iota"][:])
    i64d = const.tile([128, 64], BF16)
    nc.sync.dma_start(out=i64d[:], in_=ins["i64d"][:])
    id1f = const.tile([128, 128], FP32)
    nc.sync.dma_start(out=id1f[:], in_=ins["id1f"][:])
    xt = const.tile([128, 32 * 96], BF16)
    nc.sync.dma_start(out=xt[:], in_=ins["xt"][:])
    w1t = const.tile([96, 512], BF16)
    nc.sync.dma_start(out=w1t[:], in_=ins["w1t"][:])
    w2t = const.tile([128, 384], BF16)
    nc.sync.dma_start(out=w2t[:], in_=ins["w2t"][:])
    cwd = const.tile([128, 1536], BF16)
    nc.sync.dma_start(out=cwd[:], in_=ins["cwd"][:])
    cwp = const.tile([128, 512], BF16)
    nc.sync.dma_start(out=cwp[:], in_=ins["cwp"][:])
    ones1 = const.tile([128, 1], BF16)
    nc.gpsimd.memset(ones1[:], 1)
    b1c = b2d = None
    if has_b1:
        b1c = const.tile([128, 4], FP32)
        nc.sync.dma_start(out=b1c[:], in_=ins["b1c"][:])
    if has_b2:
        b2d = const.tile([128, 192], BF16)
        nc.sync.dma_start(out=b2d[:], in_=ins["b2d"][:])

    P = (const, ohp, gsb, psb, gps, tps, pps, cps, xt, etr, w1t, w2t, cwd,
         cwp, i64d, iota, id1f, ones1, b1c, b2d, y_d)
    # sequential emission per group (the Tile scheduler reorders globally;
    # measured faster than explicitly software-pipelined emission)
    q0 = 0
    for gs in GROUPS:
        st = _phase_a(nc, P, q0, gs, has_b1, has_b2)
        _phase_b(nc, P, st, q0, gs, has_b1, has_b2)
        _phase_c(nc, P, st, q0, gs, has_b1, has_b2)
        q0 += gs

    ctx.close()


# ---------------- host side ----------------

def _prep_consts(W1, b1, W2, b2, conv_w):
    bf = ml_dtypes.bfloat16
    w1t = np.ascontiguousarray(W1.T).astype(bf)                    # [96, 512]
    w2t = np.ascontiguousarray(
        W2.T.reshape(4, 128, 96).transpose(1, 0, 2).reshape(128, 384)
    ).astype(bf)
    # cwd[i, (k, oc, o_lo)] = conv_w[oc*128+o_lo, i, k], duplicated rows
    base = np.ascontiguousarray(
        conv_w.transpose(1, 2, 0).reshape(64, 3 * 4 * 128))
    cwd = np.concatenate([base, base], axis=0).astype(bf)          # [128,1536]
    i64 = np.eye(64)
    i64d = np.concatenate([i64, i64], axis=0).astype(bf)           # [128, 64]
    iota = np.ascontiguousarray(
        np.broadcast_to(np.arange(64).astype(bf), (128, 64)))
    id1f = np.eye(128, dtype=np.float32)
    # cwp: taps k=0,1 stacked on rows for the packed conv matmul
    ckio = conv_w.transpose(1, 2, 0)                               # [i, k, o]
    cwp = np.concatenate([ckio[:, 0, :], ckio[:, 1, :]], axis=0).astype(bf)
    consts = dict(w1t=w1t, w2t=w2t, cwd=cwd, cwp=cwp, i64d=i64d, iota=iota,
                  id1f=id1f)
    has_b1 = bool(np.any(b1))
    has_b2 = bool(np.any(b2))
    if has_b1:
        consts["b1c"] = np.ascontiguousarray(
            b1.reshape(4, 128).T).astype(np.float32)
    if has_b2:
        b2d = np.ascontiguousarray(
            np.broadcast_to(np.tile(b2, 2).astype(bf), (128, 192)))
        consts["b2d"] = b2d
    return consts, has_b1, has_b2


_NC_CACHE = {}


def _get_nc(has_b1, has_b2):
    key = (has_b1, has_b2)
    if key in _NC_CACHE:
        return _NC_CACHE[key]
    nc = bacc.Bacc("TRN2", target_bir_lowering=False, debug=False)
    ins = {
        "xt": nc.dram_tensor("xt", [128, 32 * 96], BF16,
                             kind="ExternalInput").ap(),
        "etr": nc.dram_tensor("etr", [128, 512], BF16,
                              kind="ExternalInput").ap(),
        "w1t": nc.dram_tensor("w1t", [96, 512], BF16,
                              kind="ExternalInput").ap(),
        "w2t": nc.dram_tensor("w2t", [128, 384], BF16,
                              kind="ExternalInput").ap(),
        "cwd": nc.dram_tensor("cwd", [128, 1536], BF16,
                              kind="ExternalInput").ap(),
        "i64d": nc.dram_tensor("i64d", [128, 64], BF16,
                               kind="ExternalInput").ap(),
        "iota": nc.dram_tensor("iota", [128, 64], BF16,
                               kind="ExternalInput").ap(),
        "id1f": nc.dram_tensor("id1f", [128, 128], FP32,
                               kind="ExternalInput").ap(),
        "cwp": nc.dram_tensor("cwp", [128, 512], BF16,
                              kind="ExternalInput").ap(),
    }
    if has_b1:
        ins["b1c"] = nc.dram_tensor("b1c", [128, 4], FP32,
                                    kind="ExternalInput").ap()
    if has_b2:
        ins["b2d"] = nc.dram_tensor("b2d", [128, 192], BF16,
                                    kind="ExternalInput").ap()
    outs = {
        "y": nc.dram_tensor("y", [NPAIR, 128, 768], BF16,
                            kind="ExternalOutput").ap(),
    }
    with tile.TileContext(nc) as tc:
        build_gcn_kernel(tc, outs, ins, has_b1, has_b2)
    nc.compile()
    _NC_CACHE[key] = nc
    return nc


def kernel(x, edge_index, W1, b1, W2, b2, conv_w, _trace=False):
    bf = ml_dtypes.bfloat16
    x = np.asarray(x)
    edge_index = np.asarray(edge_index)
    consts, has_b1, has_b2 = _prep_consts(
        np.asarray(W1), np.asarray(b1), np.asarray(W2), np.asarray(b2),
        np.asarray(conv_w))
    nc = _get_nc(has_b1, has_b2)

    in_maps = []
    for cid in range(N_CORES):
        sl = slice(cid * G, (cid + 1) * G)
        m = dict(consts)
        xc = np.asarray(x[sl])                       # [64, 96, 64]
        m["xt"] = np.ascontiguousarray(
            xc.reshape(32, 2, 96, 64).transpose(1, 3, 0, 2).reshape(
                128, 32 * 96)).astype(bf)
        ec = np.asarray(edge_index[sl])              # [64, 2, 512]
        m["etr"] = np.ascontiguousarray(
            ec.reshape(32, 2, 2, 4, 128).transpose(4, 0, 3, 2, 1).reshape(
                128, 512)).astype(bf)
        in_maps.append(m)

    res = run_bass_kernel_spmd(nc, in_maps, core_ids=list(range(N_CORES)),
                               trace=_trace)
    parts = []
    for cid in range(N_CORES):
        arr = np.asarray(res.results[cid]["y"])      # [32, 128, 768] bf16
        yc = arr.reshape(32, 128, 2, 4, 96).transpose(0, 2, 4, 3, 1)
        parts.append(yc.reshape(G, 96, 512).astype(np.float32))
    y = np.concatenate(parts, axis=0)
    if _trace:
        kernel.last_results = res
    return y
